# revision 42
# baseline (speedup 1.0000x reference)
"""Per-neuron grouped MLP (conv-style) kernel for Trainium2, 8 NeuronCores.

Math (per group d):  h = x[:, d, :] @ W1[d].T + b1[d]; g = gelu(h); out[:, d] = g @ W2[d] + b2[d]
  x: [B=512, D=2048, M=128], W1: [D, H=128, M], b1: [D, H], W2: [D, H], b2: [D]

Strategy (v5) — the kernel is ScalarE-bound: gelu on ACT runs at
1 elem/lane/cycle @1.2GHz regardless of dtype (HW-measured), so
B*D_LOC*H/128 = 131072 elems/partition set a 109.2us/core streaming
floor. In a gapless back-to-back stream the per-instruction dispatch
overhead pipelines to ~zero, and the measured kernel (~107-112us
median) sits at that floor; every other engine is hidden under the
gelu stream.

  - Shard on D: each of 8 cores owns D_LOC = 256 independent per-neuron MLPs.
  - x is quantized to int8 on host with a per-(d,m) scale folded into W1
    (W1'[m,d,h] = W1[d,h,m] * sx[d,m]); nc.gpsimd.dma_start casts
    int8 -> fp16 in the SDMA datapath, so HBM traffic for x is 1 byte/elem
    while the matmul runs in fp16 on integer-valued activations
    (rel err ~7e-3, gate 2e-2).
  - DRAM layouts are [M, D_LOC, *] so every per-supergroup DMA reads a
    contiguous per-partition chunk.
  - Supergroups of 12 d's (divisible by 3 for ACT triples and by 4 for
    MM2 quads; one leading 4-sg fills the pipeline fast). Per triple of
    3 d's: 3 MM1 matmuls into one [H, 3B] fp32 psum tile (3 banks,
    bufs=2), ONE Gelu (exact erf) psum -> fp16 g_sg in SBUF.
  - After the sg's last gelu: per quad c (d = D0+3j+c), 4 packed MM2s
    with tile_position=(0,32j) run concurrently in PE column groups ->
    psum rows {0,32,64,96}; DVE copies psum -> fp16 o_sb; one
    strided-partition DMA writes outT[D0:D0+12] (row order (j,c) == d).
    b2 and the fp32 upcast happen on host.
  - PSUM budget: p_tri 2x3 + p_one 1x1 + ps2 1x1 = 8 banks exactly.
  - Software pipelining at triple granularity (MM1 of unit u+1 emitted
    before the gelu of unit u) keeps ACT streaming gaplessly; a dummy
    1-col gelu hoists the ~2.7us ACT_TABLE_LOAD under the initial DMAs.
  - Variants measured and rejected: quad-level ACT in 4-bank tiles
    (Tile's whole-tile dependency tracking serializes the next MM1
    behind the DVE read of the shared tile: 165us vs 125us); fp8
    anywhere (3.6% rms error fails the gate); gelu on DVE/GPSIMD
    (no LUT; polynomial needs ~10 passes at worse throughput).
"""

import numpy as np

B, D, M, H = 512, 2048, 128, 128
N_CORES = 8
D_LOC = D // N_CORES  # 256
QUAD = 4     # d's per MM2 packing group
TRI = 3      # d's per psum1/ACT batch in 12-sgs
PAIR = 2     # d's per ACT batch in the last (early-drain) supergroup
SUPER = 12   # d's per super-group: one x DMA, one w1 DMA, one out DMA

X_INT8 = True  # False: ship x as fp16 (no quantization)

# DVE-gelu hybrid: offload ~15% of gelu triples from the saturated ScalarE
# to the idle VectorE using gelu(h) = 0.5h + E(h), E even, E approximated by
# a tail-weighted least-squares sextic in u = h^2 (max err 5e-3 on |h|<1.95,
# 6.4e-4 end-to-end through the W2 reduction). 6 DVE passes via fused
# scalar_tensor_tensor Horner steps.
GC1, GC3, GC5 = 0.3978004, -0.06072855, 0.00537447
# 11 measured a statistical wash vs 0 on HW (118.8/110.5 vs 115.1/112.3
# min/median us): DVE per-op DRAIN serialization eats the predicted gain.
# 0 = pure-ACT gelu (the validated best configuration).
N_DVE_TRI = 0

_NC_CACHE = {}


def build_nc(bias_mode: bool, x_int8: bool = X_INT8, reps: int = 1):
    key = (bias_mode, x_int8, reps)
    if key in _NC_CACHE:
        return _NC_CACHE[key]

    import concourse.bacc as bacc
    import concourse.mybir as mybir
    import concourse.tile as tile

    f32 = mybir.dt.float32
    f16 = mybir.dt.float16
    xdt = mybir.dt.int8 if x_int8 else f16
    GELU = mybir.ActivationFunctionType.Gelu

    nc = bacc.Bacc("TRN2", target_bir_lowering=False, debug=False, num_devices=N_CORES)
    xT = nc.dram_tensor("xT", [M, D_LOC, B], xdt, kind="ExternalInput").ap()
    w1T = nc.dram_tensor("w1T", [M, D_LOC, H], f16, kind="ExternalInput").ap()
    w2T = nc.dram_tensor("w2T", [H, D_LOC], f16, kind="ExternalInput").ap()
    b1T = nc.dram_tensor("b1T", [H, D_LOC], f32, kind="ExternalInput").ap()
    outT = nc.dram_tensor("outT", [D_LOC, B], f16, kind="ExternalOutput").ap()

    with (
        tile.TileContext(nc) as tc,
        tc.tile_pool(name="singles", bufs=1) as singles,
        tc.tile_pool(name="xp", bufs=3) as xp,
        tc.tile_pool(name="wp", bufs=2) as wp,
        tc.tile_pool(name="gp", bufs=3) as gp,
        tc.tile_pool(name="vp", bufs=2) as vp,
        tc.tile_pool(name="op", bufs=4) as op_pool,
        tc.tile_pool(name="ps1", bufs=2, space="PSUM") as ps1,
        tc.tile_pool(name="ps2", bufs=2, space="PSUM") as ps2,
    ):
        w2_sb = singles.tile([H, D_LOC], f16)
        nc.sync.dma_start(out=w2_sb[:], in_=w2T[:])
        b1_sb = None
        if bias_mode:
            b1_sb = singles.tile([H, D_LOC], f32)
            nc.sync.dma_start(out=b1_sb[:], in_=b1T[:])
        # Dummy 1-col gelu with no input deps: hoists the ~2.7us
        # ACT_TABLE_LOAD for the gelu set under the initial DMAs.
        warm = singles.tile([128, 1], f32)
        nc.gpsimd.memset(warm[:], 0.0)
        nc.scalar.activation(warm[:], warm[:], GELU)

        for _rep in range(reps):
            _body_loop(nc, tc, bias_mode, f16, f32, GELU,
                       xT, w1T, outT, w2_sb, b1_sb,
                       xp, wp, gp, vp, op_pool, ps1, ps2, mybir)

    nc.compile()
    _NC_CACHE[key] = nc
    return nc


def _sg_plan():
    """(D0, size) list: one leading 4-sg (single-d ACTs, fast pipeline
    fill after a 256KB DMA), then 21 supergroups of 12 (triple ACTs)."""
    sizes = [4] + [SUPER] * ((D_LOC - 4) // SUPER)
    assert sum(sizes) == D_LOC
    out, d0 = [], 0
    for s in sizes:
        out.append((d0, s))
        d0 += s
    return out


def _body_loop(nc, tc, bias_mode, f16, f32, GELU, xT, w1T, outT, w2_sb, b1_sb,
               xp, wp, gp, vp, op_pool, ps1, ps2, mybir):
    MULT = mybir.AluOpType.mult
    ADD = mybir.AluOpType.add

    sgs = _sg_plan()
    last_sgi = len(sgs) - 1
    # unit list: (sgi, u, n_d) — n_d MM1s + one gelu per unit.
    # The LAST supergroup runs pair units with contiguous-quad d-mapping
    # (quad c = d 4c..4c+3 = pairs 2c,2c+1) and per-quad g tiles, so each
    # MM2 quad drains as soon as its two gelus land: the post-last-gelu
    # tail shrinks to one quad, which also shrinks the PE-FIFO bubble
    # that delays the next execution's first MM1 in back-to-back runs.
    units = []
    for sgi, (D0, size) in enumerate(sgs):
        if sgi == last_sgi and not bias_mode:
            units += [(sgi, u, PAIR) for u in range(size // PAIR)]
        elif size % TRI == 0:
            units += [(sgi, u, TRI) for u in range(size // TRI)]
        else:
            # small supergroups run as pairs in the double-buffered 3-bank
            # psum tag: singles on a bufs=1 tag serialize MM1 behind ACT
            units += [(sgi, u, PAIR) for u in range(size // PAIR)]
    lastq = {}  # quad index -> per-quad g tile (last supergroup)

    # triples offloaded to the DVE-polynomial gelu: triple u=1 of every
    # other 12-supergroup (spread evenly so DVE latency hides under the
    # sg's remaining ACT stream)
    dve_set = set()
    if not bias_mode:
        n12 = len(sgs) - 1  # sgs[0] is the 4-sg
        cands = [(sgi, 1) for sgi in range(1, len(sgs), 2)]
        cands += [(sgi, 2) for sgi in range(2, len(sgs), 2)]
        dve_set = set(cands[:N_DVE_TRI])

    sg_state = {}   # sgi -> (x_sb, w1_sb, o_sb, g_sb)
    sg_dve = {}     # sgi -> (u_star, g_dve tile)

    def emit_mm1(ui):
        """Stage 1: (DMA loads at supergroup start) + n_d MM1 matmuls."""
        sgi, u, n_d = units[ui]
        D0, size = sgs[sgi]
        if u == 0:
            x_sb = xp.tile([M, size, B], f16, name=f"x_{size}")
            nc.gpsimd.dma_start(out=x_sb[:], in_=xT[:, D0 : D0 + size, :])
            w1_sb = wp.tile([M, size, H], f16, name=f"w1_{size}")
            nc.sync.dma_start(out=w1_sb[:], in_=w1T[:, D0 : D0 + size, :])
            o_sb = op_pool.tile([128, size // QUAD, B], f16, name=f"o_{size}")
            g_sb = gp.tile([H, size, B], f16, name=f"g_{size}")
            sg_state[sgi] = (x_sb, w1_sb, o_sb, g_sb)
        x_sb, w1_sb, _, _ = sg_state[sgi]
        # pair tiles share the 3-bank "p_3" tag slots: PSUM stays at 8 banks
        p1 = ps1.tile([H, n_d * B], f32, name="p_1" if n_d == 1 else "p_3",
                      bufs=1 if n_d == 1 else 2)
        for k in range(n_d):
            dl = n_d * u + k  # local d index within the supergroup
            nc.tensor.matmul(
                p1[:, k * B : (k + 1) * B],
                lhsT=w1_sb[:, dl, :],
                rhs=x_sb[:, dl, :],
                start=True,
                stop=True,
            )
        return p1

    def emit_consume(ui, p1):
        """Stage 2: gelu; after the sg's last unit: MM2 quads -> DVE -> DMA."""
        sgi, u, n_d = units[ui]
        D0, size = sgs[sgi]
        NQ = size // QUAD
        _, _, o_sb, g_sb = sg_state[sgi]
        if sgi == len(sgs) - 1 and not bias_mode:
            # early-drain last supergroup: pair gelus into per-quad g tiles,
            # MM2 quad c (contiguous d = D0+4c+j) fires after its 2nd pair
            c, half = u // 2, u % 2
            if half == 0:
                g_q = gp.tile([H, QUAD, B], f16, name="g_q4")
                lastq[c] = g_q
            g_q = lastq[c]
            nc.scalar.activation(g_q[:, 2 * half : 2 * half + 2, :], p1[:], GELU)
            if half == 0:
                return
            del lastq[c]
            p2 = ps2.tile([128, B], f32)
            for j in range(QUAD):
                dd = D0 + 4 * c + j
                nc.tensor.matmul(
                    p2[32 * j : 32 * j + 1, :],
                    lhsT=w2_sb[:, dd : dd + 1],
                    rhs=g_q[:, j, :],
                    start=True,
                    stop=True,
                    tile_position=(0, 32 * j),
                )
            o_q = op_pool.tile([128, 1, B], f16, name="o_q4")
            nc.vector.tensor_copy(o_q[:, 0, :], p2[:])
            nc.sync.dma_start(
                out=outT[D0 + 4 * c : D0 + 4 * c + 4, :], in_=o_q[0::32, :, :]
            )
            if u == size // PAIR - 1:
                del sg_state[sgi]
            return
        gsl = g_sb[:, n_d * u : n_d * (u + 1), :]
        if bias_mode:
            for k in range(n_d):
                dd = D0 + n_d * u + k
                nc.scalar.activation(
                    gsl[:, k, :],
                    p1[:, k * B : (k + 1) * B],
                    GELU,
                    bias=b1_sb[:, dd : dd + 1],
                )
        elif (sgi, u) in dve_set:
            # DVE sextic gelu: g = 0.5h + u2*(C1 + C3*u2 + C5*u2^2), u2 = h^2.
            # Only tensor_tensor (2x f16) / tensor_scalar (4x f16) ops.
            FD = n_d * B
            h16 = vp.tile([H, FD], f16, name="v_h")
            nc.vector.tensor_copy(h16[:], p1[:])
            u2 = vp.tile([H, FD], f16, name="v_u")
            nc.vector.tensor_tensor(u2[:], h16[:], h16[:], MULT)
            t = vp.tile([H, FD], f16, name="v_t")
            nc.vector.tensor_scalar(t[:], u2[:], GC5, GC3, MULT, ADD)
            t2 = vp.tile([H, FD], f16, name="v_t2")
            nc.vector.tensor_tensor(t2[:], u2[:], t[:], MULT)
            t3 = vp.tile([H, FD], f16, name="v_t3")
            nc.vector.tensor_scalar(t3[:], t2[:], 1.0, GC1, MULT, ADD)
            ep = vp.tile([H, FD], f16, name="v_e")
            nc.vector.tensor_tensor(ep[:], u2[:], t3[:], MULT)
            r = vp.tile([H, FD], f16, name="v_r")
            nc.vector.tensor_scalar(r[:], h16[:], 0.5, None, MULT)
            g_dve = vp.tile([H, n_d, B], f16, name="v_g")
            nc.vector.tensor_tensor(
                g_dve[:].rearrange("h d b -> h (d b)"), ep[:], r[:], ADD
            )
            sg_dve[sgi] = (u, g_dve)
        else:
            nc.scalar.activation(gsl[:], p1[:], GELU)
        if n_d * (u + 1) != size:
            return
        # supergroup's gelus complete: MM2 quads (col-tiled, concurrent),
        # DVE copies, one strided out DMA. Quad c: d = D0 + NQ*j + c;
        # the DVE-offloaded triple u* covers d = 3u*..3u*+2 == (j=u*, all c).
        del sg_state[sgi]
        u_star, g_dve = sg_dve.pop(sgi, (None, None))
        for c in range(NQ):
            p2 = ps2.tile([128, B], f32)
            for j in range(QUAD):
                dl = NQ * j + c
                rhs = g_dve[:, c, :] if j == u_star else g_sb[:, dl, :]
                nc.tensor.matmul(
                    p2[32 * j : 32 * j + 1, :],
                    lhsT=w2_sb[:, D0 + dl : D0 + dl + 1],
                    rhs=rhs,
                    start=True,
                    stop=True,
                    tile_position=(0, 32 * j),
                )
            nc.vector.tensor_copy(o_sb[:, c, :], p2[:])
        nc.sync.dma_start(
            out=outT[D0 : D0 + size, :], in_=o_sb[0::32, :, :]
        )

    # 1-deep software pipeline at unit granularity: PE runs MM1(u+1)
    # while ACT consumes unit u.
    prev = emit_mm1(0)
    for ui in range(len(units)):
        if ui + 1 < len(units):
            nxt = emit_mm1(ui + 1)
        emit_consume(ui, prev)
        prev = nxt if ui + 1 < len(units) else None


def prepare_in_maps(x, W1, b1, W2, x_int8: bool = X_INT8):
    """Host-side shard + transpose (+ int8 quantization). 8 per-core dicts."""
    x = np.asarray(x, dtype=np.float32)
    W1 = np.asarray(W1, dtype=np.float32)
    b1 = np.asarray(b1, dtype=np.float32)
    W2 = np.asarray(W2, dtype=np.float32)

    in_maps = []
    for k in range(N_CORES):
        sl = slice(k * D_LOC, (k + 1) * D_LOC)
        xk = x[:, sl, :]  # [B, D_LOC, M]
        w1k = W1[sl]      # [D_LOC, H, M]
        if x_int8:
            sx = np.abs(xk).max(axis=0) / 127.0          # [D_LOC, M]
            sx = np.maximum(sx, 1e-12)
            xq = np.rint(xk / sx[None]).astype(np.int8)  # [B, D_LOC, M]
            xT_k = np.ascontiguousarray(xq.transpose(2, 1, 0))          # [M, D_LOC, B]
            w1s = w1k * sx[:, None, :]                   # [D_LOC, H, M] * sx[d,m]
        else:
            xT_k = np.ascontiguousarray(
                xk.transpose(2, 1, 0), dtype=np.float16
            )
            w1s = w1k
        w1T_k = np.ascontiguousarray(w1s.transpose(2, 0, 1), dtype=np.float16)  # [M, D_LOC, H]
        w2T_k = np.ascontiguousarray(W2[sl].T, dtype=np.float16)
        b1T_k = np.ascontiguousarray(b1[sl].T, dtype=np.float32)
        in_maps.append({"xT": xT_k, "w1T": w1T_k, "w2T": w2T_k, "b1T": b1T_k})
    return in_maps


def assemble_output(results, b2):
    outT_full = np.concatenate([r["outT"] for r in results], axis=0)  # [D, B] f16
    out = outT_full.T.astype(np.float32)  # [B, D]
    b2 = np.asarray(b2, dtype=np.float32)
    if np.any(b2):
        out = out + b2[None, :]
    return np.ascontiguousarray(out)


def kernel(pre_activation_history, W1, b1, W2, b2):
    from concourse.bass_utils import run_bass_kernel_spmd

    b1 = np.asarray(b1, dtype=np.float32)
    bias_mode = bool(np.any(b1))
    nc = build_nc(bias_mode)
    in_maps = prepare_in_maps(pre_activation_history, W1, b1, W2)
    res = run_bass_kernel_spmd(nc, in_maps, core_ids=list(range(N_CORES)))
    return assemble_output(res.results, b2)
